# revision 33
# baseline (speedup 1.0000x reference)
"""Cross-attention kernel for 8 Trainium2 NeuronCores.

Reference computation (per batch element a):
  K = X @ Wk, Q = L @ Wq, V = X @ Wv          (each head uses a full 256-dim slice)
  S_i = Q_i @ K_i^T / sqrt(32); P = softmax(S); A_i = P_i @ V_i
  out = concat_i(A_i) @ Wu + bu
Sharding: core c = 2*a + hg handles batch a and head-group hg (4 heads);
the two partial outputs per batch element are summed on the host, which
also adds the bias.

Perf design: the TRN2 PE runs at 2.4 GHz only after ~3us of continuous
execution (1.2 GHz before, 0.65 GHz cold), so the whole kernel is built
to keep the PE streaming without a single dependency stall:

  * Softmax normalization is DEFERRED past the attention matmul.  A^T
    tiles hold unnormalized sums of exp()*V; the output matmul
    accumulates each head into its own PSUM slice, and the per-head
    1/sum enters as a per-partition scalar on the PSUM->SBUF eviction
    (out rows are y, matching the per-y denominators).
  * Denominators: exp tiles are tree-summed on DVE/GPSIMD (off the PE
    path), then reduced across partitions by tiny N=1 PE matmuls
    (lhsT=acc block, rhs=ones column) directly into a transposed
    [y-part, 1] PSUM layout -- no gpsimd all-reduce, no [128,512]
    reciprocals, and the PE never waits on any of it.

All matmuls run as float32r (fp32 storage; 1 row/cycle at N>=256).

Device layouts (per core) -- contraction dim on SBUF partitions, so no
on-device transposes:
  XT, LT           [256, 1024]  x^T / latent^T   (host pre-transposes)
  WK, WQ, WV       [256, 1024];  WU [1024, 256]
  KT = (X@WK)^T    [1024(n), 1024(s)]  via lhsT=WK-tile, rhs=XT
  QT = (L@WQ)^T    [1024(n), 1024(y)]  via lhsT=WQ-tile, rhs=LT
  V  = X@WV        [1024(s), 1024(n)]  via lhsT=XT-tile, rhs=WV
  S^T_i            [b, y] psum via lhsT=KT-tile, rhs=QT        (per head i)
  P^T_i = exp(.)   [b, y] sbuf, ACT exp with scale fused
  acc_i            [b, y] running sum of P^T tiles (DVE+GPSIMD tree)
  sumsT_i          [y_block, 1] psum via lhsT=acc-block, rhs=ones
  A^T_i (unnorm)   [c, y] psum via lhsT=V-tile, rhs=P^T; plain eviction
  O_h psums        [y, e] per head; evicted * recT_h, then summed
"""

import math
import sys

import ml_dtypes
import numpy as np

sys.path.insert(0, "/opt/trn_rl_repo")

import concourse.bass as bass  # noqa: E402
import concourse.mybir as mybir  # noqa: E402
from concourse import bacc  # noqa: E402
from concourse.bass_utils import run_bass_kernel_spmd  # noqa: E402
from concourse.tile import TileContext  # noqa: E402

F32 = mybir.dt.float32
F32R = mybir.dt.float32r
BF16 = mybir.dt.bfloat16
EXP = mybir.ActivationFunctionType.Exp

B, S, E = 4, 1024, 256          # batch, seq, embed
HEADS = 8                        # total heads; each head dim = E (source quirk)
N_CORES = 8
HG_HEADS = 4                     # heads per head-group (per core)
NH = HG_HEADS * E                # projection columns per core = 1024
SCALE = 1.0 / math.sqrt(E // HEADS)   # 1/sqrt(32)

P = 128                          # SBUF partitions
NT = NH // P                     # 8 partition tiles of the projection dim
ST = S // P                      # 8 partition tiles of the seq dim
NCH = 512                        # matmul moving-dim chunk
SCH = S // NCH                   # 2 chunks of 512 over seq
YB = NCH // P                    # 4 y-blocks of 128 per chunk

_CACHE = {}

import os as _os
SCRATCH_BUFS = int(_os.environ.get("K_SCRATCH", "10"))
SC_BUFS = int(_os.environ.get("K_SC", "4"))
PA_BUFS = int(_os.environ.get("K_PA", "3"))


def _build():
    nc = bacc.Bacc(target_bir_lowering=False)

    XT = nc.dram_tensor("XT", [E, S], BF16, kind="ExternalInput")
    LT = nc.dram_tensor("LT", [E, S], BF16, kind="ExternalInput")
    WK = nc.dram_tensor("WK", [E, NH], BF16, kind="ExternalInput")
    WQ = nc.dram_tensor("WQ", [E, NH], BF16, kind="ExternalInput")
    WV = nc.dram_tensor("WV", [E, NH], BF16, kind="ExternalInput")
    WU = nc.dram_tensor("WU", [NH, E], BF16, kind="ExternalInput")
    O = nc.dram_tensor("O", [S, E], F32, kind="ExternalOutput")

    ET = E // P  # 2 partition tiles of the embed (contraction) dim

    with TileContext(nc) as tc:
        with tc.tile_pool(name="persist", bufs=1) as pp, \
             tc.tile_pool(name="scratch", bufs=SCRATCH_BUFS) as sp, \
             tc.tile_pool(name="small", bufs=2) as mp, \
             tc.tile_pool(name="psum", bufs=1, space="PSUM") as ps:

            # ---- phase 0: load inputs (chunked so compute starts early) ----
            def alloc_in(nm):
                return [sp.tile([P, S], BF16, tag="big", name=f"{nm}{t}")
                        for t in range(ET)]

            xt, wk, wv, lt, wq = (alloc_in(n) for n in ("xt", "wk", "wv", "lt", "wq"))

            # spread DMA triggers across the four idle engine DGE queues so
            # trigger issue (~0.6us each) and transfers parallelize instead
            # of serializing behind one Sync queue
            dmae = [nc.sync, nc.gpsimd, nc.scalar]
            dma_rr = [0]

            def dma_issue(out, in_):
                eng = dmae[dma_rr[0] % len(dmae)]
                dma_rr[0] += 1
                eng.dma_start(out=out, in_=in_)

            def dma_chunk(tiles, dram, e, c):
                dma_issue(tiles[e][:, c * NCH:(c + 1) * NCH],
                          dram[e * P:(e + 1) * P, c * NCH:(c + 1) * NCH])

            # issue order: operands of the first KT groups first.  xt[.][:,
            # 0:256] land first (the first projection group is split in two
            # 256-wide halves so a 64KB transfer, not 128KB, gates the
            # stream start on the cold DMA queue)
            HC = NCH // 2
            dma_issue(xt[0][:, 0:HC], XT[0:P, 0:HC])
            dma_issue(wk[0][:, 0:NCH], WK[0:P, 0:NCH])
            dma_issue(xt[1][:, 0:HC], XT[P:2 * P, 0:HC])
            dma_issue(wk[1][:, 0:NCH], WK[P:2 * P, 0:NCH])
            dma_issue(xt[0][:, HC:NCH], XT[0:P, HC:NCH])
            dma_issue(xt[1][:, HC:NCH], XT[P:2 * P, HC:NCH])
            for e in range(ET):
                dma_chunk(wk, WK, e, 1)
            for e in range(ET):
                dma_chunk(xt, XT, e, 1)
            for c in range(SCH):
                for e in range(ET):
                    dma_chunk(wv, WV, e, c)
            for c in range(SCH):
                for e in range(ET):
                    dma_chunk(lt, LT, e, c)
            for c in range(SCH):
                for e in range(ET):
                    dma_chunk(wq, WQ, e, c)
            wu_all = pp.tile([P, NT * E], BF16, tag="wu", name="wu_all")
            nc.sync.dma_start(out=wu_all[:].rearrange("p (t e) -> p t e", t=NT),
                              in_=WU.rearrange("(t p) e -> p t e", p=P))
            wu = [wu_all[:, t * E:(t + 1) * E] for t in range(NT)]

            # ones columns for the partition-reduction matmuls (bf16: fp32r
            # trips walrus ISA restrictions at tiny N, fp32 is dual-pass)
            ones = pp.tile([P, 2], BF16, tag="ones", name="ones")
            nc.vector.memset(ones[:], 1.0)

            # p-state warmup: the PE only reaches 2.4 GHz after ~3us of
            # continuous execution; these junk matmuls (no DMA deps) keep it
            # busy through the input-DMA window so the projection stream
            # starts at full clock.  N=512 so each covers real time.
            junk = pp.tile([P, NCH], BF16, tag="junk", name="junk")
            nc.vector.memset(junk[:], 0.5)
            warm = ps.tile([2, NCH], F32, tag="sums", bufs=1, name="warm")
            for w in range(7):
                nc.tensor.matmul(warm[:], ones[:], junk[:],
                                 start=True, stop=True)

            # ---- phase 1: projections KT, QT (transposed), V (natural) ----
            kt, qt, v = [], [], []
            for nt in range(NT):
                kt.append(pp.tile([P, S], BF16, tag=f"kt{nt}", name=f"kt{nt}"))
                qt.append(pp.tile([P, S], BF16, tag=f"qt{nt}", name=f"qt{nt}"))
                v.append(pp.tile([P, S], BF16, tag=f"v{nt}", name=f"v{nt}"))

            # alternate PSUM->SBUF evictions between DVE and ACT so neither
            # engine gates the PE during the projection phase
            evict_ctr = [0]

            def evict(dst_ap, src_ap):
                evict_ctr[0] += 1
                if evict_ctr[0] % 2 == 0:
                    nc.vector.tensor_copy(dst_ap, src_ap)
                else:
                    nc.scalar.activation(dst_ap, src_ap,
                                         mybir.ActivationFunctionType.Copy)

            def proj(dst, lhs_tiles, lhs_cols, rhs_tiles, nt, c, nm):
                sl = bass.ts(c, NCH)
                pk = ps.tile([P, NCH], F32, tag="pA", bufs=PA_BUFS, name=f"p{nm}{nt}{c}")
                for e in range(ET):
                    nc.tensor.matmul(pk[:], lhs_tiles[e][:, lhs_cols],
                                     rhs_tiles[e][:, sl],
                                     start=(e == 0), stop=(e == ET - 1))
                evict(dst[nt][:, sl], pk[:])

            def proj_half(dst, lhs_tiles, lhs_cols, rhs_tiles, nt, half, nm):
                sl = slice(half * HC, (half + 1) * HC)
                pk = ps.tile([P, HC], F32, tag="pA", bufs=PA_BUFS,
                             name=f"ph{nm}{nt}{half}")
                for e in range(ET):
                    nc.tensor.matmul(pk[:], lhs_tiles[e][:, lhs_cols],
                                     rhs_tiles[e][:, sl],
                                     start=(e == 0), stop=(e == ET - 1))
                evict(dst[nt][:, sl], pk[:])

            # first projection split into 256-wide halves: its first group
            # depends only on the 64KB xt[.][:, 0:256] transfers
            proj_half(kt, wk, slice(0, P), xt, 0, 0, "k")
            proj_half(kt, wk, slice(0, P), xt, 0, 1, "k")
            for nt in range(1, NT):
                proj(kt, wk, slice(nt * P, (nt + 1) * P), xt, nt, 0, "k")
            for nt in range(NT):
                proj(kt, wk, slice(nt * P, (nt + 1) * P), xt, nt, 1, "k")
            for c in range(SCH):
                for nt in range(NT):
                    proj(v, xt, slice(nt * P, (nt + 1) * P), wv, nt, c, "v")
            for c in range(SCH):
                for nt in range(NT):
                    proj(qt, wq, slice(nt * P, (nt + 1) * P), lt, nt, c, "q")

            # ---- phase 2: attention per head ----
            at = []
            for nt in range(NT):
                at.append(pp.tile([P, S], BF16, tag=f"at{nt}", name=f"at{nt}"))

            pt_h = {}
            acc_h = {}
            acc2_h = {}
            accbf_h = {}
            psT_h = {}
            recT_h = {}

            def st_group(h, c, bt):
                c0 = 2 * h
                sl = bass.ts(c, NCH)
                pt, acc = pt_h[h], acc_h[h]
                pss = ps.tile([P, NCH], F32, tag="sc", bufs=SC_BUFS,
                              name=f"pss{h}{bt}{c}")
                for cj in range(2):
                    nc.tensor.matmul(pss[:], kt[c0 + cj][:, bt * P:(bt + 1) * P],
                                     qt[c0 + cj][:, sl],
                                     start=(cj == 0), stop=(cj == 1))
                nc.scalar.activation(pt[bt][:, sl], pss[:], EXP, scale=SCALE)
                # denominator accumulation pipelined with the exps; the
                # two half-trees run on DVE and the (otherwise idle) GPSIMD
                acc2 = acc2_h[h]
                if bt == 1:
                    nc.vector.tensor_add(acc[:, sl], pt[0][:, sl], pt[1][:, sl])
                elif bt in (2, 3):
                    nc.vector.tensor_add(acc[:, sl], acc[:, sl], pt[bt][:, sl])
                elif bt == 5:
                    nc.gpsimd.tensor_add(acc2[:, sl], pt[4][:, sl], pt[5][:, sl])
                elif bt in (6, 7):
                    nc.gpsimd.tensor_add(acc2[:, sl], acc2[:, sl], pt[bt][:, sl])

            def sums_finish(h, c):
                # final combine of the two half-trees (DVE, off the PE path);
                # bf16 output so the partition-reduce matmuls are single-pass
                sl = bass.ts(c, NCH)
                nc.vector.tensor_add(accbf_h[h][:, sl], acc_h[h][:, sl],
                                     acc2_h[h][:, sl])

            def sums_mms(h, c):
                # partition-reduce acc into transposed [y,1] layout: 4 tiny
                # N=2 bf16 matmuls (lhsT = 128-wide acc block, rhs = ones)
                for yb in range(YB):
                    col = c * YB + yb
                    y0 = c * NCH + yb * P
                    nc.tensor.matmul(psT_h[h][:, 2 * col:2 * col + 2],
                                     accbf_h[h][:, y0:y0 + P], ones[:],
                                     start=True, stop=True)

            def rec_finish(h):
                # every second psT column holds a sums copy (N=2 matmuls)
                nc.vector.reciprocal(recT_h[h][:], psT_h[h][:, 0:2 * SCH * YB:2])

            def at_group(h, c, ct):
                # A^T accumulation over b; plain eviction (normalization is
                # deferred to the output phase)
                sl = bass.ts(c, NCH)
                vsl = slice(h * E + ct * P, h * E + (ct + 1) * P)
                pa = ps.tile([P, NCH], F32, tag="pA", bufs=PA_BUFS,
                             name=f"pa{h}{ct}{c}")
                for bt in range(ST):
                    nc.tensor.matmul(pa[:], v[bt][:, vsl], pt_h[h][bt][:, sl],
                                     start=(bt == 0), stop=(bt == ST - 1))
                evict(at[2 * h + ct][:, sl], pa[:])

            def head_alloc(h):
                # bf16 + a 16-deep ring: two heads of exp tiles coexist, so
                # head h+1's score evictions never wait on head h's A^T reads
                pt_h[h] = [sp.tile([P, S], BF16, tag="ptb", bufs=16,
                                   name=f"pt{h}{bt}")
                           for bt in range(ST)]
                acc_h[h] = mp.tile([P, S], F32R, tag="sacc", name=f"sacc{h}")
                acc2_h[h] = mp.tile([P, S], F32R, tag="sacc2", name=f"sacc2{h}")
                accbf_h[h] = mp.tile([P, S], BF16, tag="saccbf", name=f"saccbf{h}")
                psT_h[h] = ps.tile([P, 2 * SCH * YB], F32, tag="sums", bufs=1,
                                   name=f"psT{h}", space="PSUM")
                recT_h[h] = mp.tile([P, SCH * YB], F32, tag="recT", bufs=4,
                                    name=f"recT{h}")

            def out_tile(yt):
                # per-head output psums (two heads per PSUM tile), combined
                # with the per-partition 1/sums scales on eviction
                ysl = slice(yt * P, (yt + 1) * P)
                poA = ps.tile([P, 2 * E], F32, tag="sc", bufs=SC_BUFS,
                              name=f"poA{yt}")
                poB = ps.tile([P, 2 * E], F32, tag="sc", bufs=SC_BUFS,
                              name=f"poB{yt}")
                for h in range(HG_HEADS):
                    po = poA if h < 2 else poB
                    esl = slice((h % 2) * E, (h % 2) * E + E)
                    for ctj in range(2):
                        ht = 2 * h + ctj
                        nc.tensor.matmul(po[:, esl], at[ht][:, ysl], wu[ht],
                                         start=(ctj == 0), stop=(ctj == 1))
                # scaled evictions: ACT takes heads 0/2, DVE heads 1/3
                eh = []
                for h in range(HG_HEADS):
                    po = poA if h < 2 else poB
                    esl = slice((h % 2) * E, (h % 2) * E + E)
                    t = mp.tile([P, E], F32, tag="osb", bufs=8,
                                name=f"osb{yt}h{h}")
                    rsc = recT_h[h][:, yt:yt + 1]
                    if h % 2 == 0:
                        nc.scalar.mul(t[:], po[:, esl], rsc)
                    else:
                        nc.vector.tensor_scalar_mul(t[:], po[:, esl], rsc)
                    eh.append(t)
                s0 = mp.tile([P, E], F32, tag="osb", bufs=8, name=f"os0{yt}")
                nc.gpsimd.tensor_add(s0[:], eh[0][:], eh[1][:])
                nc.vector.tensor_add(eh[3][:], eh[2][:], eh[3][:])
                nc.vector.tensor_add(s0[:], s0[:], eh[3][:])
                nc.sync.dma_start(out=O[yt * P:(yt + 1) * P, :], in_=s0[:])

            # cross-head software pipeline: head h's chunk-1 A^T groups are
            # woven into head h+1's chunk-0 score stream, so the PE always
            # has matmul work while ACT drains the exp queue.  The sums
            # matmuls for chunk c are woven one chunk later (their acc tree
            # finishes early in the next chunk).
            for h in range(HG_HEADS):
                head_alloc(h)
                for bt in range(ST):
                    st_group(h, 0, bt)
                    if h > 0:
                        if bt == 2:
                            at_group(h - 1, 1, 0)
                        elif bt == 4:
                            sums_mms(h - 1, 1)
                        elif bt == 5:
                            at_group(h - 1, 1, 1)
                if h > 0:
                    rec_finish(h - 1)
                sums_finish(h, 0)
                for bt in range(ST):
                    st_group(h, 1, bt)
                    if bt == 2:
                        at_group(h, 0, 0)
                    elif bt == 4:
                        sums_mms(h, 0)
                    elif bt == 5:
                        at_group(h, 0, 1)
                sums_finish(h, 1)
            LAST = HG_HEADS - 1

            # ---- phase 3: head-3's final A^T + sums first (the out-phase
            # evictions consume recT[3], and engine queues are in-order, so
            # rec_finish(LAST) must precede every out_tile), then outputs ----
            at_group(LAST, 1, 0)
            at_group(LAST, 1, 1)
            sums_mms(LAST, 1)
            rec_finish(LAST)
            for yt in range(ST):
                out_tile(yt)

    nc.compile()
    return nc


def kernel(batch, latent, Wk, Wq, Wv, Wu, bu):
    batch = np.asarray(batch, dtype=np.float32)
    latent = np.asarray(latent, dtype=np.float32)
    Wk = np.asarray(Wk, dtype=np.float32)
    Wq = np.asarray(Wq, dtype=np.float32)
    Wv = np.asarray(Wv, dtype=np.float32)
    Wu = np.asarray(Wu, dtype=np.float32)
    bu = np.asarray(bu, dtype=np.float32)

    if "nc" not in _CACHE:
        _CACHE["nc"] = _build()
    nc = _CACHE["nc"]

    bf = ml_dtypes.bfloat16
    in_maps = []
    for core in range(N_CORES):
        a, hg = core // 2, core % 2
        cols = slice(hg * NH, (hg + 1) * NH)
        in_maps.append({
            "XT": np.ascontiguousarray(batch[a].T).astype(bf),
            "LT": np.ascontiguousarray(latent[a].T).astype(bf),
            "WK": np.ascontiguousarray(Wk[:, cols]).astype(bf),
            "WQ": np.ascontiguousarray(Wq[:, cols]).astype(bf),
            "WV": np.ascontiguousarray(Wv[:, cols]).astype(bf),
            "WU": np.ascontiguousarray(Wu[cols, :]).astype(bf),
        })

    _CACHE["in_maps"] = in_maps
    res = run_bass_kernel_spmd(nc, in_maps, core_ids=list(range(N_CORES)))

    out = np.empty((B, S, E), dtype=np.float32)
    for a in range(B):
        out[a] = res.results[2 * a]["O"] + res.results[2 * a + 1]["O"] + bu
    return out
